# revision 37
# baseline (speedup 1.0000x reference)
"""Trainium2 Bass kernel: 8-head transformer encoder layer (B=8, S=1024,
D=300, Dh=512, H=8), data-parallel over batch across 8 NeuronCores.

Matmul operands are bf16 (PE: 1 cycle/column vs 4 for fp32); accumulation
stays fp32 in PSUM and LN/softmax stats stay fp32. The score LayerNorm
divides by the score std (~2.4e3), so bf16's absolute rounding errors
normalize away (~7e-3 end-to-end vs the 2e-2 gate).

Per core (one batch element):
  qT/kT = Wp @ x^T  (heads contiguous via host-side weight-row permute)
  v     = x @ Wp^T   (ones-augmented: column 64 of each head slice is 1.0,
                      so the AV matmul also produces the softmax denominator)
  stats: sum_t e^2 = q^T G q with G = sum_t k k^T (the mean^2 term is
         ~1e-3 of M2 and is dropped); c = gamma/sqrt(M2/(S-1) + D*eps)
  qc = qT * bcast(c)  (c pre-folded into q, via K=1 ones matmul broadcast)
  scores computed TRANSPOSED per (head, t-tile): eT = kT-chunk^T @ qc
         -> exp(eT) (ACT, FD=1024) writes pT directly; no PE transposes
  AV: [heads; r]^T = v_aug^T-chunks @ pT ; aT = heads^T * bcast(1/r)
  attention is software-pipelined: scores/exp(hq) are emitted before
  AV(hq-1) so the in-order PE queue never stalls waiting on ACT
  x1 = a @ WO ; x2 = LN(x1 + x) ; x2T via PE transpose
  h1T = relu(W1-as-lhsT @ x2T + b1) ; h2 = h1T-chunks @ W2
  out = LN(h2 + b2 + x2)
"""

import numpy as np
import ml_dtypes

import concourse.bass as bass
import concourse.tile as tile
from concourse import bacc, mybir
from concourse.bass_utils import run_bass_kernel_spmd
from concourse.masks import make_identity

F32 = mybir.dt.float32
BF = mybir.dt.bfloat16
NPBF = ml_dtypes.bfloat16
AF = mybir.ActivationFunctionType

B, S, D, DH, H, DHD = 8, 1024, 300, 512, 8, 64
DF = 4 * D  # 1200
EPS = 1e-8
NCORES = 8

DP, DFP = 384, 1280  # D/DF zero-padded to 128 multiples (full-partition streams)
J_CHUNKS = [(0, 128), (128, 128), (256, 128)]
M_CHUNKS = [(i * 128, 128) for i in range(10)]
N_ST = S // 128  # 8 s-tiles
N_SH = S // 512  # 2 s-halves

TRACE = False
_cache = {}
_last_results = None


def _build_nc(dbg=False):
    nc = bacc.Bacc("TRN2", debug=False)

    # consolidated inputs: each [128, W] DMA costs ~8 serialized descriptor
    # instructions on its issue queue regardless of W, so everything is
    # packed into a few wide tensors and spread over the three DGE queues
    WB0 = S + DH          # 1536: [xt | wq] — needed first
    WB1 = DH * 2 + DFP    # 2304: [wk | wv | w1]
    xd = nc.dram_tensor("x", [S, D], F32, kind="ExternalInput").ap()
    bigw0d = nc.dram_tensor("bigw0", [DP, WB0], BF, kind="ExternalInput").ap()
    bigw1d = nc.dram_tensor("bigw1", [DP, WB1], BF, kind="ExternalInput").ap()
    bigmd = nc.dram_tensor("bigm", [128, 14 * D], BF, kind="ExternalInput").ap()
    fb1d = nc.dram_tensor("fb1", [128, 10], F32, kind="ExternalInput").ap()
    fb2d = nc.dram_tensor("fb2", [D], F32, kind="ExternalInput").ap()
    gald = nc.dram_tensor("gal", [12], F32, kind="ExternalInput").ap()
    outd = nc.dram_tensor("out", [S, D], F32, kind="ExternalOutput").ap()
    if dbg:
        dqT = nc.dram_tensor("dqT", [DH, S], BF, kind="ExternalOutput").ap()
        dkT = nc.dram_tensor("dkT", [DH, S], BF, kind="ExternalOutput").ap()
        dv = nc.dram_tensor("dv", [S, H, DHD + 1], BF, kind="ExternalOutput").ap()
        dc8 = nc.dram_tensor("dc8", [128, H, N_ST], F32, kind="ExternalOutput").ap()
        dpT = nc.dram_tensor("dpT", [128, 8, S], BF, kind="ExternalOutput").ap()
        daT = nc.dram_tensor("daT", [DH, S], BF, kind="ExternalOutput").ap()
        dx2 = nc.dram_tensor("dx2", [S, D], F32, kind="ExternalOutput").ap()
        dh1 = nc.dram_tensor("dh1", [DFP, S], BF, kind="ExternalOutput").ap()

    with tile.TileContext(nc) as tc:
        with (
            tc.tile_pool(name="wts", bufs=1) as wts,
            tc.tile_pool(name="work", bufs=1) as work,
            tc.tile_pool(name="sm", bufs=8) as sm,
            tc.tile_pool(name="ps", bufs=1, space="PSUM") as ps,
        ):
            # ------------- input loads, spread over 3 DGE queues -------------
            bigw0_sb, bigw1_sb = [], []
            dge = [nc.sync, nc.scalar, nc.gpsimd]
            for jc, (j0, jn) in enumerate(J_CHUNKS):
                t = wts.tile([128, WB0], BF, tag=f"bigw0_{jc}")
                dge[jc].dma_start(out=t[:jn, :], in_=bigw0d[j0 : j0 + jn, :])
                bigw0_sb.append(t)
            for jc, (j0, jn) in enumerate(J_CHUNKS):
                t = wts.tile([128, WB1], BF, tag=f"bigw1_{jc}")
                dge[jc].dma_start(out=t[:jn, :], in_=bigw1d[j0 : j0 + jn, :])
                bigw1_sb.append(t)
            xt_sb = [t[:, 0:S] for t in bigw0_sb]
            wq_sb = [t[:, S : S + DH] for t in bigw0_sb]
            wk_sb = [t[:, 0:DH] for t in bigw1_sb]
            wv_sb = [t[:, DH : 2 * DH] for t in bigw1_sb]
            w1_sb = [t[:, 2 * DH :] for t in bigw1_sb]

            gal_bc = wts.tile([128, 12], F32, tag="gal")
            nc.gpsimd.dma_start(
                out=gal_bc,
                in_=bass.AP(tensor=gald.tensor, offset=gald.offset,
                            ap=[[0, 128]] + list(gald.ap)),
            )
            ga_bc = [gal_bc[:, h : h + 1] for h in range(H)]
            g1_bc, b1_bc = gal_bc[:, 8:9], gal_bc[:, 9:10]
            g2_bc, b2_bc = gal_bc[:, 10:11], gal_bc[:, 11:12]

            bigm_sb = wts.tile([128, 14 * D], BF, tag="bigm")
            nc.scalar.dma_start(out=bigm_sb, in_=bigmd)
            wo_sb = [bigm_sb[:, it * D : (it + 1) * D] for it in range(4)]
            w2_sb = [bigm_sb[:, (4 + mt) * D : (5 + mt) * D] for mt in range(10)]

            # x natural: [128, 8, 300] (partition = s % 128) — needed at LN1
            x_sb = wts.tile([128, N_ST, D], F32, tag="x")
            nc.sync.dma_start(out=x_sb, in_=xd.rearrange("(n p) d -> p n d", p=128))

            fb1p = wts.tile([128, 10], F32, tag="fb1p")
            nc.gpsimd.dma_start(out=fb1p, in_=fb1d)
            fb1_sb = [fb1p[:, mt : mt + 1] for mt in range(10)]
            fb2_bc = wts.tile([128, D], F32, tag="fb2")
            nc.gpsimd.dma_start(
                out=fb2_bc,
                in_=bass.AP(tensor=fb2d.tensor, offset=fb2d.offset,
                            ap=[[0, 128]] + list(fb2d.ap)),
            )

            # ---------------- constants ----------------
            ident = wts.tile([128, 128], BF, tag="ident")
            make_identity(nc, ident)
            identf = wts.tile([128, 128], F32, tag="identf")
            make_identity(nc, identf)

            ones1 = wts.tile([1, 128], BF, tag="ones1")
            nc.vector.memset(ones1, 1.0)
            dummy = wts.tile([128, 1], BF, tag="dummy")
            nc.vector.memset(dummy, 1.0)

            eps_a = wts.tile([128, 1], F32, tag="eps_a")  # D*EPS (score LN)
            nc.vector.memset(eps_a, D * EPS)
            eps_l = wts.tile([128, 1], F32, tag="eps_l")  # EPS (x LNs)
            nc.vector.memset(eps_l, EPS)

            # ---------------- phase 1: projections ----------------
            qT = [work.tile([128, S], BF, tag="big4k", bufs=14, name=f"qT{i}") for i in range(4)]
            kT = [work.tile([128, S], BF, tag="big4k", bufs=14, name=f"kT{i}") for i in range(4)]
            v_sb = [work.tile([128, H, DHD + 1], BF, tag="v2k", bufs=10, name=f"v{i}") for i in range(N_ST)]

            psn = [0]

            def pstile(shape=(128, 512)):
                # alternate PSUM tags so pipelined phases get 4 buffers
                psn[0] += 1
                tag = "e" if psn[0] % 2 else "e2"
                return ps.tile(list(shape), F32, tag=tag, bufs=2, name=f"pp{psn[0] % 4}")

            def proj(dst, w):
                # dst[dt][:, sh-chunk] = (w-cols)^T @ x^T
                for dt in range(4):
                    for sh in range(N_SH):
                        pp = pstile()
                        for jc, (j0, jn) in enumerate(J_CHUNKS):
                            nc.tensor.matmul(
                                pp,
                                lhsT=w[jc][:jn, dt * 128 : (dt + 1) * 128],
                                rhs=xt_sb[jc][:jn, sh * 512 : (sh + 1) * 512],
                                start=(jc == 0),
                                stop=(jc == 2),
                            )
                        nc.vector.tensor_copy(
                            out=dst[dt][:, sh * 512 : (sh + 1) * 512], in_=pp
                        )

            # qT first, then kn+G, so the PE-light stats phase can overlap
            # the kT/v projections that follow it
            proj(qT, wq_sb)
            G_ps = ps.tile([64, H, DHD], F32, tag="pt", bufs=2)
            nc.vector.memset(G_ps, 0.0)
            for st in range(N_ST):
                # k in natural [t, d] layout, for G_h = sum_t k_t k_t^T
                pk = pstile()
                for jc, (j0, jn) in enumerate(J_CHUNKS):
                    nc.tensor.matmul(
                        pk,
                        lhsT=xt_sb[jc][:jn, st * 128 : (st + 1) * 128],
                        rhs=wk_sb[jc][:jn, :],
                        start=(jc == 0),
                        stop=(jc == 2),
                    )
                kn = work.tile([128, 512], BF, tag="v2k", bufs=10, name="kn")
                nc.vector.tensor_copy(out=kn, in_=pk)
                for h in range(H):
                    nc.tensor.matmul(
                        G_ps[:, h, :],
                        lhsT=kn[:, h * DHD : (h + 1) * DHD],
                        rhs=kn[:, h * DHD : (h + 1) * DHD],
                        start=False,
                        stop=(st == N_ST - 1),
                        skip_group_check=True,
                    )
            if dbg:
                for i in range(4):
                    nc.sync.dma_start(out=dqT[i * 128 : (i + 1) * 128, :], in_=qT[i])

            # ---------------- phase 2a: analytic score stats ----------------
            # sum_t e^2 = q^T G q (the mean^2 correction is ~1e-3 of M2 and
            # is dropped); c = gamma / sqrt(M2/(S-1) + D*eps)
            G_sb = wts.tile([128, H, DHD], BF, tag="gsb")
            nc.vector.tensor_copy(out=G_sb[0:64, :, :], in_=G_ps)
            nc.sync.dma_start(out=G_sb[64:128, :, :], in_=G_sb[0:64, :, :])

            c8all = wts.tile([128, H, N_ST], F32, tag="c8all")
            for hq in range(4):
                sums2 = ps.tile([128, N_ST, 2], F32, tag="e", bufs=2)
                for sh in range(N_SH):
                    y2 = ps.tile([128, 512], F32, tag="e2", bufs=2)
                    for j in range(2):
                        hp = j * 64
                        nc.tensor.matmul(
                            y2[hp : hp + 64, :],
                            lhsT=G_sb[hp : hp + 64, hq * 2 + j, :],
                            rhs=qT[hq][hp : hp + 64, sh * 512 : (sh + 1) * 512],
                            start=True,
                            stop=True,
                        )
                    z_sb = sm.tile([128, 512], BF, tag="z", bufs=2)
                    for j in range(2):
                        hp = j * 64
                        nc.vector.tensor_tensor(
                            out=z_sb[hp : hp + 64, :],
                            in0=qT[hq][hp : hp + 64, sh * 512 : (sh + 1) * 512],
                            in1=y2[hp : hp + 64, :],
                            op=mybir.AluOpType.mult,
                        )
                    for st4 in range(4):
                        st = sh * 4 + st4
                        for j in range(2):
                            hp = j * 64
                            nc.tensor.matmul(
                                sums2[:, st, j : j + 1],
                                lhsT=z_sb[hp : hp + 64, st4 * 128 : (st4 + 1) * 128],
                                rhs=dummy[hp : hp + 64, :],
                                start=True,
                                stop=True,
                            )
                for j in range(2):
                    h = hq * 2 + j
                    sd8 = sm.tile([128, N_ST], F32, tag="sd8", bufs=2)
                    nc.scalar.activation(
                        out=sd8, in_=sums2[:, :, j], func=AF.Sqrt,
                        bias=eps_a, scale=1.0 / (S - 1),
                    )
                    nc.vector.reciprocal(out=c8all[:, h, :], in_=sd8)
                    nc.vector.tensor_scalar_mul(
                        c8all[:, h, :], c8all[:, h, :], ga_bc[h]
                    )
            if dbg:
                nc.sync.dma_start(out=dc8, in_=c8all)

            # kT projections emitted after the stats: the PE-dense stream
            # overlaps the stats phase's DVE/ACT tail. The v projection is
            # deferred into the attention loop (PE filler while ACT runs the
            # first head-pair's exps).
            proj(kT, wk_sb)

            def v_proj():
                for st in range(N_ST):
                    pp = pstile()
                    for jc, (j0, jn) in enumerate(J_CHUNKS):
                        nc.tensor.matmul(
                            pp,
                            lhsT=xt_sb[jc][:jn, st * 128 : (st + 1) * 128],
                            rhs=wv_sb[jc][:jn, :],
                            start=(jc == 0),
                            stop=(jc == 2),
                        )
                    nc.vector.tensor_copy(
                        out=v_sb[st][:, :, 0:DHD],
                        in_=pp.rearrange("p (h d) -> p h d", h=H),
                    )
                    nc.vector.memset(v_sb[st][:, :, DHD : DHD + 1], 1.0)

            if dbg:
                for i in range(4):
                    nc.sync.dma_start(out=dkT[i * 128 : (i + 1) * 128, :], in_=kT[i])

            # c8 transposed to free-dim order: c8t[h*8+st, p] = c8[p, h, st],
            # then flattened to a single row via identity-column row-selects
            # so the per-head broadcast matmul runs from base partition 0
            c8t_ps = ps.tile([64, 128], F32, tag="e", bufs=2)
            nc.tensor.transpose(
                c8t_ps, c8all.rearrange("p h n -> p (h n)"), identf
            )
            c8t_sb = wts.tile([64, 128], BF, tag="c8t")
            nc.vector.tensor_copy(out=c8t_sb, in_=c8t_ps)
            c8row_all = wts.tile([1, H * S], BF, tag="c8row")
            for h in range(H):
                crow_ps = ps.tile([1, S], F32, tag="e2", bufs=2)
                for st in range(N_ST):
                    hst = h * N_ST + st
                    nc.tensor.matmul(
                        crow_ps[0:1, st * 128 : (st + 1) * 128],
                        lhsT=ident[0:64, hst : hst + 1],
                        rhs=c8t_sb,
                        start=True,
                        stop=True,
                    )
                nc.vector.tensor_copy(
                    out=c8row_all[0:1, h * S : (h + 1) * S], in_=crow_ps
                )

            # ---------------- phase 2b: attention ----------------
            aT = [work.tile([128, S], BF, tag="big4k", bufs=14, name=f"aT{i}") for i in range(4)]

            # qc = qT * bcast(c8): c folded into q before the scores; all
            # four head-pairs precomputed so attention never waits on DVE
            qc_t = []
            for hq in range(4):
                cbc_ps = ps.tile([128, S], F32, tag="e2", bufs=2)
                for j in range(2):
                    hp = j * 64
                    h = hq * 2 + j
                    for half in range(2):
                        nc.tensor.matmul(
                            cbc_ps[hp : hp + 64, half * 512 : (half + 1) * 512],
                            lhsT=ones1[0:1, 0:64],
                            rhs=c8row_all[0:1, h * S + half * 512 : h * S + (half + 1) * 512],
                            start=True,
                            stop=True,
                        )
                qc = work.tile([128, S], BF, tag="big4k", bufs=14, name=f"qc{hq}")
                nc.vector.tensor_tensor(
                    out=qc, in0=qT[hq], in1=cbc_ps, op=mybir.AluOpType.mult
                )
                qc_t.append(qc)

            pending = []

            def flush_pending():
                # deferred per-head normalization: by now the reciprocal has
                # long finished, so the rbc matmul never stalls the PE
                while pending:
                    dst_hq, dst_sh, j, avsb, rbf = pending.pop(0)
                    hp = j * 64
                    rbc_ps = ps.tile([128, 512], F32, tag="e", bufs=2)
                    nc.tensor.matmul(rbc_ps, lhsT=ones1, rhs=rbf, start=True, stop=True)
                    rbc_sb = sm.tile([128, 512], BF, tag="rbc", bufs=2)
                    nc.vector.tensor_copy(out=rbc_sb, in_=rbc_ps)
                    nc.vector.tensor_tensor(
                        out=aT[dst_hq][hp : hp + 64, dst_sh * 512 : (dst_sh + 1) * 512],
                        in0=avsb,
                        in1=rbc_sb[0:DHD, :],
                        op=mybir.AluOpType.mult,
                    )

            def do_av(hq, pT2, sh_list=(0, 1)):
                for sh in sh_list:
                    for j in range(2):
                        h = hq * 2 + j
                        av_ps = ps.tile([DHD + 1, 512], F32, tag="pt", bufs=2)
                        for tj in range(8):
                            nc.tensor.matmul(
                                av_ps,
                                lhsT=v_sb[tj][:, h, :],
                                rhs=pT2[j][:, tj, sh * 512 : (sh + 1) * 512],
                                start=(tj == 0),
                                stop=(tj == 7),
                            )
                        # heads to SBUF right away (releases the PSUM bank);
                        # denominator row via fast-approx reciprocal
                        avsb = sm.tile([DHD, 512], BF, tag="avsb", bufs=4)
                        nc.vector.tensor_copy(out=avsb, in_=av_ps[0:DHD, :])
                        rrow_f = sm.tile([1, 512], F32, tag="rrowf", bufs=2)
                        nc.vector.tensor_copy(out=rrow_f, in_=av_ps[DHD : DHD + 1, :])
                        rinv = sm.tile([1, 512], F32, tag="rinv", bufs=2)
                        nc.vector.reciprocal_approx_fast(out=rinv, in_=rrow_f)
                        rbf = sm.tile([1, 512], BF, tag="rbf", bufs=2)
                        nc.vector.tensor_copy(out=rbf, in_=rinv)
                        pending.append((hq, sh, j, avsb, rbf))
                    flush_pending()

            prev = None
            for hq in range(4):
                qc = qc_t[hq]
                pT2 = [
                    work.tile([128, 8, S], BF, tag="pt16k", bufs=4, name=f"pT{j}")
                    for j in range(2)
                ]
                for tt in range(8):
                    for j in range(2):
                        hp = j * 64
                        eT = ps.tile([128, S], F32, tag="e2", bufs=2, name=f"eT{j}")
                        for half in range(2):
                            nc.tensor.matmul(
                                eT[:, half * 512 : (half + 1) * 512],
                                lhsT=kT[hq][hp : hp + 64, tt * 128 : (tt + 1) * 128],
                                rhs=qc[hp : hp + 64, half * 512 : (half + 1) * 512],
                                start=True,
                                stop=True,
                            )
                        nc.scalar.activation(
                            out=pT2[j][:, tt, :], in_=eT, func=AF.Exp,
                            bias=0.0, scale=1.0,
                        )
                if dbg and hq == 0:
                    nc.sync.dma_start(out=dpT, in_=pT2[0])
                if hq == 0:
                    v_proj()
                    if dbg:
                        for i in range(N_ST):
                            nc.sync.dma_start(
                                out=dv[i * 128 : (i + 1) * 128, :, :], in_=v_sb[i]
                            )
                if prev is not None:
                    do_av(*prev)
                prev = (hq, pT2)

            # ---------------- phase 3: WO + residual + LN1 ----------------
            x2f_sb = [work.tile([128, D], F32, tag="v2kf", bufs=8, name=f"x2f_{i}") for i in range(N_ST)]
            x2T = [work.tile([128, S], BF, tag="big4k", bufs=14, name=f"x2T{i}") for i in range(3)]
            nc.vector.memset(x2T[2], 0.0)
            LCORR = float(D) / float(D - 1)

            def layer_norm(dst, src_ps, res_tiles, g_bc, b_bc):
                xr = sm.tile([128, D], F32, tag="xr", bufs=3)
                nc.vector.tensor_add(xr, src_ps, res_tiles[0])
                for rt in res_tiles[1:]:
                    nc.vector.tensor_add(xr, xr, rt)
                stats = sm.tile([128, 6], F32, tag="lstats", bufs=4)
                nc.vector.bn_stats(out=stats, in_=xr)
                mv = sm.tile([128, 2], F32, tag="lmv", bufs=4)
                nc.vector.bn_aggr(out=mv, in_=stats)
                sd = sm.tile([128, 1], F32, tag="lsd", bufs=4)
                nc.scalar.activation(
                    out=sd, in_=mv[:, 1:2], func=AF.Sqrt, bias=eps_l, scale=LCORR
                )
                rstd = sm.tile([128, 1], F32, tag="lrstd", bufs=4)
                nc.vector.reciprocal(out=rstd, in_=sd)
                grstd = sm.tile([128, 1], F32, tag="lgr", bufs=4)
                nc.vector.tensor_mul(grstd, rstd, g_bc)
                nc.vector.tensor_scalar(
                    out=dst,
                    in0=xr,
                    scalar1=mv[:, 0:1],
                    scalar2=grstd,
                    op0=mybir.AluOpType.subtract,
                    op1=mybir.AluOpType.mult,
                )
                nc.vector.tensor_scalar_add(dst, dst, b_bc)

            def wo_ln1(st_list):
                for st in st_list:
                    x1_ps = ps.tile([128, D], F32, tag="e", bufs=2)
                    for it in range(4):
                        nc.tensor.matmul(
                            x1_ps,
                            lhsT=aT[it][:, st * 128 : (st + 1) * 128],
                            rhs=wo_sb[it],
                            start=(it == 0),
                            stop=(it == 3),
                        )
                    layer_norm(x2f_sb[st], x1_ps, [x_sb[:, st, :]], g1_bc, b1_bc)
                    xt_ps = ps.tile([128, 4, 128], F32, tag="pt", bufs=2)
                    for jc, (j0, jn) in enumerate([(0, 128), (128, 128), (256, 44)]):
                        nc.tensor.transpose(
                            xt_ps[:jn, jc, :], x2f_sb[st][:, j0 : j0 + jn], identf
                        )
                    for jc, (j0, jn) in enumerate([(0, 128), (128, 128), (256, 44)]):
                        nc.vector.tensor_copy(
                            out=x2T[jc][:jn, st * 128 : (st + 1) * 128],
                            in_=xt_ps[:jn, jc, :],
                        )

            # drain the last head pair one s-half at a time, interleaving the
            # WO/LN1 tiles that become ready
            do_av(prev[0], prev[1], (0,))
            wo_ln1(range(0, 4))
            do_av(prev[0], prev[1], (1,))
            wo_ln1(range(4, 8))

            if dbg:
                for i in range(4):
                    nc.sync.dma_start(out=daT[i * 128 : (i + 1) * 128, :], in_=aT[i])
                for i in range(N_ST):
                    nc.sync.dma_start(out=dx2[i * 128 : (i + 1) * 128, :],
                                      in_=x2f_sb[i])

            # ---------------- phase 4: FFN + LN2 ----------------
            # FFN1(sh) then immediately FFN2+LN2 for that s-half, so the LN2
            # chains overlap the other half's FFN1; relu alternates between
            # ACT and DVE to halve the activation-engine serial time
            h1T = [work.tile([128, S], BF, tag="big4k", bufs=14, name=f"h1T{i}") for i in range(10)]
            for sh in range(N_SH):
                for mt, (m0, msz) in enumerate(M_CHUNKS):
                    h1_ps = pstile()
                    for jc, (j0, jn) in enumerate(J_CHUNKS):
                        nc.tensor.matmul(
                            h1_ps[:msz, :],
                            lhsT=w1_sb[jc][:jn, m0 : m0 + msz],
                            rhs=x2T[jc][:jn, sh * 512 : (sh + 1) * 512],
                            start=(jc == 0),
                            stop=(jc == 2),
                        )
                    h1o = h1T[mt][:msz, sh * 512 : (sh + 1) * 512]
                    if mt % 2 == 0:
                        nc.scalar.activation(
                            out=h1o, in_=h1_ps[:msz, :], func=AF.Relu,
                            bias=fb1_sb[mt][:msz, :], scale=1.0,
                        )
                    else:
                        nc.vector.tensor_scalar(
                            out=h1o,
                            in0=h1_ps[:msz, :],
                            scalar1=fb1_sb[mt][:msz, :],
                            scalar2=0.0,
                            op0=mybir.AluOpType.add,
                            op1=mybir.AluOpType.max,
                        )
                for st in range(sh * 4, sh * 4 + 4):
                    h2_ps = ps.tile([128, D], F32, tag="e2", bufs=2)
                    for mt, (m0, msz) in enumerate(M_CHUNKS):
                        nc.tensor.matmul(
                            h2_ps,
                            lhsT=h1T[mt][:msz, st * 128 : (st + 1) * 128],
                            rhs=w2_sb[mt][:msz, :],
                            start=(mt == 0),
                            stop=(mt == 9),
                        )
                    o_sb = sm.tile([128, D], F32, tag="o", bufs=2)
                    layer_norm(o_sb, h2_ps, [fb2_bc, x2f_sb[st]], g2_bc, b2_bc)
                    dge[st % 2].dma_start(
                        out=outd[st * 128 : (st + 1) * 128, :], in_=o_sb
                    )
            if dbg:
                for mt, (m0, msz) in enumerate(M_CHUNKS):
                    nc.sync.dma_start(out=dh1[m0 : m0 + msz, :], in_=h1T[mt][:msz, :])

    nc.compile()
    return nc


def _get_nc():
    if "nc" not in _cache:
        _cache["nc"] = _build_nc()
    return _cache["nc"]


def prep_in_maps(x, WQ, WK, WV, WO, W1, b1, W2, b2, gamma_a, beta_a,
                 gamma1, beta1, gamma2, beta2):
    f = np.float32
    x = np.asarray(x, f)

    def perm(W):
        # head h -> contiguous rows [h*64, (h+1)*64)
        return np.asarray(W, f).reshape(DHD, H, D).transpose(1, 0, 2).reshape(DH, D)

    def padr(a, rows, cols=None):
        out = np.zeros((rows, cols or a.shape[1]), f)
        out[: a.shape[0], : a.shape[1]] = a
        return out

    def tobf(a):
        return np.ascontiguousarray(a.astype(NPBF))

    wq_t = padr(perm(WQ).T, DP)
    wk_t = padr(perm(WK).T, DP)
    wv_t = padr(perm(WV).T, DP)
    w1 = padr(np.asarray(W1, f), DP, DFP)
    # bigm: [wo rows as 4 col-blocks | w2 rows as 10 col-blocks] on 128 rows
    wo4 = np.asarray(WO, f).reshape(4, 128, D).transpose(1, 0, 2).reshape(128, 4 * D)
    w2p = padr(np.asarray(W2, f), DFP)
    w210 = w2p.reshape(10, 128, D).transpose(1, 0, 2).reshape(128, 10 * D)
    bigm = tobf(np.concatenate([wo4, w210], axis=1))
    b1p = np.zeros(1280, f)
    b1p[:DF] = np.asarray(b1, f)
    fb1 = np.ascontiguousarray(b1p.reshape(10, 128).T)
    fb2 = np.ascontiguousarray(np.asarray(b2, f))
    # beta_a drops out of softmax (per-row constant shift); the 1/sqrt(D)
    # score scale cancels inside the score LayerNorm: softmax(g*LN(e/sqrt(D)))
    # == softmax(g/sqrt(var(e) + D*eps) * e), so gamma is used unscaled and
    # D*eps replaces eps on-device.
    gal = np.concatenate([
        np.asarray(gamma_a, f).reshape(H),
        np.asarray([gamma1, beta1, gamma2, beta2], f),
    ]).astype(f)

    bigw1 = tobf(np.concatenate([wk_t, wv_t, w1], axis=1))
    shared = {"bigw1": bigw1, "bigm": bigm, "fb1": fb1, "fb2": fb2, "gal": gal}
    in_maps = []
    for b in range(B):
        xb = np.ascontiguousarray(x[b])
        xt = padr(np.ascontiguousarray(xb.T), DP)
        bigw0 = tobf(np.concatenate([xt, wq_t], axis=1))
        in_maps.append({"x": xb, "bigw0": bigw0, **shared})
    return in_maps


def kernel(**inputs):
    global _last_results
    in_maps = prep_in_maps(**inputs)
    nc = _get_nc()
    res = run_bass_kernel_spmd(nc, in_maps, core_ids=list(range(NCORES)), trace=TRACE)
    _last_results = res
    return np.stack([res.results[b]["out"] for b in range(B)], axis=0)


# revision 46
# speedup vs baseline: 1.1839x; 1.1839x over previous
"""Trainium2 Bass kernel: 8-head transformer encoder layer (B=8, S=1024,
D=300, Dh=512, H=8), data-parallel over batch across 8 NeuronCores.

Matmul operands are bf16 (PE: 1 cycle/column vs 4 for fp32); accumulation
stays fp32 in PSUM and LN/softmax stats stay fp32. The score LayerNorm
divides by the score std (~2.4e3), so bf16's absolute rounding errors
normalize away (~7e-3 end-to-end vs the 2e-2 gate).

Per core (one batch element):
  qT/kT = Wp @ x^T  (heads contiguous via host-side weight-row permute)
  v     = x @ Wp^T   (ones-augmented: column 64 of each head slice is 1.0,
                      so the AV matmul also produces the softmax denominator)
  stats: sum_t e^2 = q^T G q with G = sum_t k k^T (the mean^2 term is
         ~1e-3 of M2 and is dropped); c = gamma/sqrt(M2/(S-1) + D*eps)
  qc = qT * bcast(c)  (c pre-folded into q, via K=1 ones matmul broadcast)
  scores computed TRANSPOSED per (head, t-tile): eT = kT-chunk^T @ qc
         -> exp(eT) (ACT, FD=1024) writes pT directly; no PE transposes
  AV: [heads; r]^T = v_aug^T-chunks @ pT ; aT = heads^T * bcast(1/r)
  attention is software-pipelined: scores/exp(hq) are emitted before
  AV(hq-1) so the in-order PE queue never stalls waiting on ACT
  x1 = a @ WO ; x2 = LN(x1 + x) ; x2T via PE transpose
  h1T = relu(W1-as-lhsT @ x2T + b1) ; h2 = h1T-chunks @ W2
  out = LN(h2 + b2 + x2)
"""

import numpy as np
import ml_dtypes

import concourse.bass as bass
import concourse.tile as tile
from concourse import bacc, mybir
from concourse.bass_utils import run_bass_kernel_spmd
from concourse.masks import make_identity

F32 = mybir.dt.float32
BF = mybir.dt.bfloat16
NPBF = ml_dtypes.bfloat16
AF = mybir.ActivationFunctionType

B, S, D, DH, H, DHD = 8, 1024, 300, 512, 8, 64
DF = 4 * D  # 1200
EPS = 1e-8
NCORES = 8

DP, DFP = 384, 1280  # D/DF zero-padded to 128 multiples (full-partition streams)
J_CHUNKS = [(0, 128), (128, 128), (256, 128)]
M_CHUNKS = [(i * 128, 128) for i in range(10)]
N_ST = S // 128  # 8 s-tiles
N_SH = S // 512  # 2 s-halves

TRACE = False
_cache = {}
_last_results = None


def _build_nc(dbg=False):
    nc = bacc.Bacc("TRN2", debug=False)

    # consolidated inputs: each [128, W] DMA costs ~8 serialized descriptor
    # instructions on its issue queue regardless of W, so everything is
    # packed into a few wide tensors and spread over the three DGE queues
    WB0 = S + DH          # 1536: [xt | wq] — needed first
    WB1 = DH * 2 + DFP    # 2304: [wk | wv | w1]
    xd = nc.dram_tensor("x", [S, D], F32, kind="ExternalInput").ap()
    bigw0d = nc.dram_tensor("bigw0", [DP, WB0], BF, kind="ExternalInput").ap()
    bigw1d = nc.dram_tensor("bigw1", [DP, WB1], BF, kind="ExternalInput").ap()
    bigmd = nc.dram_tensor("bigm", [128, 14 * D], BF, kind="ExternalInput").ap()
    fb1d = nc.dram_tensor("fb1", [128, 10], F32, kind="ExternalInput").ap()
    fb2d = nc.dram_tensor("fb2", [D], F32, kind="ExternalInput").ap()
    gald = nc.dram_tensor("gal", [12], F32, kind="ExternalInput").ap()
    outd = nc.dram_tensor("out", [S, D], F32, kind="ExternalOutput").ap()
    if dbg:
        dqT = nc.dram_tensor("dqT", [DH, S], BF, kind="ExternalOutput").ap()
        dkT = nc.dram_tensor("dkT", [DH, S], BF, kind="ExternalOutput").ap()
        dv = nc.dram_tensor("dv", [S, H, DHD + 1], BF, kind="ExternalOutput").ap()
        dc8 = nc.dram_tensor("dc8", [128, H, N_ST], F32, kind="ExternalOutput").ap()
        dpT = nc.dram_tensor("dpT", [128, 8, S], BF, kind="ExternalOutput").ap()
        daT = nc.dram_tensor("daT", [DH, S], BF, kind="ExternalOutput").ap()
        dx2 = nc.dram_tensor("dx2", [S, D], F32, kind="ExternalOutput").ap()
        dh1 = nc.dram_tensor("dh1", [DFP, S], BF, kind="ExternalOutput").ap()

    with tile.TileContext(nc) as tc:
        with (
            tc.tile_pool(name="wts", bufs=1) as wts,
            tc.tile_pool(name="work", bufs=1) as work,
            tc.tile_pool(name="sm", bufs=8) as sm,
            tc.tile_pool(name="ps", bufs=1, space="PSUM") as ps,
        ):
            # ------------- input loads, spread over 3 DGE queues -------------
            bigw0_sb, bigw1_sb = [], []
            dge = [nc.sync, nc.scalar, nc.gpsimd]
            for jc, (j0, jn) in enumerate(J_CHUNKS):
                t = wts.tile([128, WB0], BF, tag=f"bigw0_{jc}")
                dge[jc].dma_start(out=t[:jn, :], in_=bigw0d[j0 : j0 + jn, :])
                bigw0_sb.append(t)
            for jc, (j0, jn) in enumerate(J_CHUNKS):
                t = wts.tile([128, WB1], BF, tag=f"bigw1_{jc}")
                dge[jc].dma_start(out=t[:jn, :], in_=bigw1d[j0 : j0 + jn, :])
                bigw1_sb.append(t)
            xt_sb = [t[:, 0:S] for t in bigw0_sb]
            wq_sb = [t[:, S : S + DH] for t in bigw0_sb]
            wk_sb = [t[:, 0:DH] for t in bigw1_sb]
            wv_sb = [t[:, DH : 2 * DH] for t in bigw1_sb]
            w1_sb = [t[:, 2 * DH :] for t in bigw1_sb]

            gal_bc = wts.tile([128, 12], F32, tag="gal")
            nc.gpsimd.dma_start(
                out=gal_bc,
                in_=bass.AP(tensor=gald.tensor, offset=gald.offset,
                            ap=[[0, 128]] + list(gald.ap)),
            )
            ga_bc = [gal_bc[:, h : h + 1] for h in range(H)]
            g1_bc, b1_bc = gal_bc[:, 8:9], gal_bc[:, 9:10]
            g2_bc, b2_bc = gal_bc[:, 10:11], gal_bc[:, 11:12]

            bigm_sb = wts.tile([128, 14 * D], BF, tag="bigm")
            nc.scalar.dma_start(out=bigm_sb, in_=bigmd)
            wo_sb = [bigm_sb[:, it * D : (it + 1) * D] for it in range(4)]
            w2_sb = [bigm_sb[:, (4 + mt) * D : (5 + mt) * D] for mt in range(10)]

            # x natural: [128, 8, 300] (partition = s % 128) — needed at LN1
            x_sb = wts.tile([128, N_ST, D], F32, tag="x")
            nc.sync.dma_start(out=x_sb, in_=xd.rearrange("(n p) d -> p n d", p=128))

            fb1p = wts.tile([128, 10], F32, tag="fb1p")
            nc.gpsimd.dma_start(out=fb1p, in_=fb1d)
            fb1_sb = [fb1p[:, mt : mt + 1] for mt in range(10)]
            fb2_bc = wts.tile([128, D], F32, tag="fb2")
            nc.gpsimd.dma_start(
                out=fb2_bc,
                in_=bass.AP(tensor=fb2d.tensor, offset=fb2d.offset,
                            ap=[[0, 128]] + list(fb2d.ap)),
            )

            # ---------------- constants ----------------
            ident = wts.tile([128, 128], BF, tag="ident")
            make_identity(nc, ident)
            identf = wts.tile([128, 128], F32, tag="identf")
            make_identity(nc, identf)

            ones1 = wts.tile([1, 128], BF, tag="ones1")
            nc.vector.memset(ones1, 1.0)
            dummy = wts.tile([128, 1], BF, tag="dummy")
            nc.vector.memset(dummy, 1.0)

            eps_a = wts.tile([128, 1], F32, tag="eps_a")  # D*EPS (score LN)
            nc.vector.memset(eps_a, D * EPS)
            eps_l = wts.tile([128, 1], F32, tag="eps_l")  # EPS (x LNs)
            nc.vector.memset(eps_l, EPS)

            # ---------------- phase 1: projections ----------------
            qT = [work.tile([128, S], BF, tag="big4k", bufs=14, name=f"qT{i}") for i in range(4)]
            kT = [work.tile([128, S], BF, tag="big4k", bufs=14, name=f"kT{i}") for i in range(4)]
            v_sb = [work.tile([128, H, DHD + 1], BF, tag="v2k", bufs=10, name=f"v{i}") for i in range(N_ST)]

            psn = [0]

            def pstile(shape=(128, 512)):
                # alternate PSUM tags so pipelined phases get 4 buffers
                psn[0] += 1
                tag = "e" if psn[0] % 2 else "e2"
                return ps.tile(list(shape), F32, tag=tag, bufs=2, name=f"pp{psn[0] % 4}")

            def proj_group(dst, w, dt, sh):
                pp = pstile()
                for jc, (j0, jn) in enumerate(J_CHUNKS):
                    nc.tensor.matmul(
                        pp,
                        lhsT=w[jc][:jn, dt * 128 : (dt + 1) * 128],
                        rhs=xt_sb[jc][:jn, sh * 512 : (sh + 1) * 512],
                        start=(jc == 0),
                        stop=(jc == 2),
                    )
                nc.vector.tensor_copy(
                    out=dst[dt][:, sh * 512 : (sh + 1) * 512], in_=pp
                )

            def proj(dst, w):
                # dst[dt][:, sh-chunk] = (w-cols)^T @ x^T
                for dt in range(4):
                    for sh in range(N_SH):
                        proj_group(dst, w, dt, sh)

            # qT first, then kn+G, so the PE-light stats phase can overlap
            # the kT/v projections that follow it
            proj(qT, wq_sb)
            # k in natural [t, d] layout, for G_h = sum_t k_t k_t^T. The G
            # accumulation for tile st-1 is emitted behind the pk matmuls of
            # tile st, so the PE never waits on the kn copy
            G_ps = ps.tile([64, H, DHD], F32, tag="pt", bufs=2)
            nc.vector.memset(G_ps, 0.0)
            kn_prev = None
            for st in range(N_ST + 1):
                if st < N_ST:
                    pk = pstile()
                    for jc, (j0, jn) in enumerate(J_CHUNKS):
                        nc.tensor.matmul(
                            pk,
                            lhsT=xt_sb[jc][:jn, st * 128 : (st + 1) * 128],
                            rhs=wk_sb[jc][:jn, :],
                            start=(jc == 0),
                            stop=(jc == 2),
                        )
                    kn = work.tile([128, 512], BF, tag="v2k", bufs=10, name="kn")
                    nc.vector.tensor_copy(out=kn, in_=pk)
                else:
                    kn = None
                if kn_prev is not None:
                    for h in range(H):
                        nc.tensor.matmul(
                            G_ps[:, h, :],
                            lhsT=kn_prev[:, h * DHD : (h + 1) * DHD],
                            rhs=kn_prev[:, h * DHD : (h + 1) * DHD],
                            start=False,
                            stop=(st == N_ST),
                            skip_group_check=True,
                        )
                kn_prev = kn
            if dbg:
                for i in range(4):
                    nc.sync.dma_start(out=dqT[i * 128 : (i + 1) * 128, :], in_=qT[i])

            # ---------------- phase 2a: analytic score stats ----------------
            # sum_t e^2 = q^T G q (the mean^2 correction is ~1e-3 of M2 and
            # is dropped); c = gamma / sqrt(M2/(S-1) + D*eps)
            G_sb = wts.tile([128, H, DHD], BF, tag="gsb")
            nc.vector.tensor_copy(out=G_sb[0:64, :, :], in_=G_ps)
            nc.sync.dma_start(out=G_sb[64:128, :, :], in_=G_sb[0:64, :, :])

            c8all = wts.tile([128, H, N_ST], F32, tag="c8all")
            for hq in range(4):
                # two kT projection groups per head-pair: dense PE streams
                # that fill the stats phase's DVE-dependency bubbles
                proj_group(kT, wk_sb, hq, 0)
                proj_group(kT, wk_sb, hq, 1)
                sums2 = ps.tile([128, N_ST, 2], F32, tag="e", bufs=2)
                for sh in range(N_SH):
                    y2 = ps.tile([128, 512], F32, tag="e2", bufs=2)
                    for j in range(2):
                        hp = j * 64
                        nc.tensor.matmul(
                            y2[hp : hp + 64, :],
                            lhsT=G_sb[hp : hp + 64, hq * 2 + j, :],
                            rhs=qT[hq][hp : hp + 64, sh * 512 : (sh + 1) * 512],
                            start=True,
                            stop=True,
                        )
                    z_sb = sm.tile([128, 512], BF, tag="z", bufs=2)
                    for j in range(2):
                        hp = j * 64
                        nc.vector.tensor_tensor(
                            out=z_sb[hp : hp + 64, :],
                            in0=qT[hq][hp : hp + 64, sh * 512 : (sh + 1) * 512],
                            in1=y2[hp : hp + 64, :],
                            op=mybir.AluOpType.mult,
                        )
                    for st4 in range(4):
                        st = sh * 4 + st4
                        for j in range(2):
                            hp = j * 64
                            nc.tensor.matmul(
                                sums2[:, st, j : j + 1],
                                lhsT=z_sb[hp : hp + 64, st4 * 128 : (st4 + 1) * 128],
                                rhs=dummy[hp : hp + 64, :],
                                start=True,
                                stop=True,
                            )
                for j in range(2):
                    h = hq * 2 + j
                    sd8 = sm.tile([128, N_ST], F32, tag="sd8", bufs=2)
                    nc.scalar.activation(
                        out=sd8, in_=sums2[:, :, j], func=AF.Sqrt,
                        bias=eps_a, scale=1.0 / (S - 1),
                    )
                    nc.vector.reciprocal(out=c8all[:, h, :], in_=sd8)
                    nc.vector.tensor_scalar_mul(
                        c8all[:, h, :], c8all[:, h, :], ga_bc[h]
                    )
            if dbg:
                nc.sync.dma_start(out=dc8, in_=c8all)

            # The v projection is deferred into the attention loop (PE filler
            # while ACT runs the first head-pair's exps).
            def v_proj():
                for st in range(N_ST):
                    pp = pstile()
                    for jc, (j0, jn) in enumerate(J_CHUNKS):
                        nc.tensor.matmul(
                            pp,
                            lhsT=xt_sb[jc][:jn, st * 128 : (st + 1) * 128],
                            rhs=wv_sb[jc][:jn, :],
                            start=(jc == 0),
                            stop=(jc == 2),
                        )
                    nc.vector.tensor_copy(
                        out=v_sb[st][:, :, 0:DHD],
                        in_=pp.rearrange("p (h d) -> p h d", h=H),
                    )
                    nc.vector.memset(v_sb[st][:, :, DHD : DHD + 1], 1.0)

            if dbg:
                for i in range(4):
                    nc.sync.dma_start(out=dkT[i * 128 : (i + 1) * 128, :], in_=kT[i])

            # c8 transposed to free-dim order: c8t[h*8+st, p] = c8[p, h, st],
            # then flattened to a single row via identity-column row-selects
            # so the per-head broadcast matmul runs from base partition 0
            c8t_ps = ps.tile([64, 128], F32, tag="e", bufs=2)
            nc.tensor.transpose(
                c8t_ps, c8all.rearrange("p h n -> p (h n)"), identf
            )
            c8t_sb = wts.tile([64, 128], BF, tag="c8t")
            nc.vector.tensor_copy(out=c8t_sb, in_=c8t_ps)
            c8row_all = wts.tile([1, H * S], BF, tag="c8row")
            for h in range(H):
                crow_ps = ps.tile([1, S], F32, tag="e2", bufs=2)
                for st in range(N_ST):
                    hst = h * N_ST + st
                    nc.tensor.matmul(
                        crow_ps[0:1, st * 128 : (st + 1) * 128],
                        lhsT=ident[0:64, hst : hst + 1],
                        rhs=c8t_sb,
                        start=True,
                        stop=True,
                    )
                nc.scalar.copy(
                    out=c8row_all[0:1, h * S : (h + 1) * S], in_=crow_ps
                )

            # ---------------- phase 2b: attention ----------------
            aT = [work.tile([128, S], BF, tag="big4k", bufs=14, name=f"aT{i}") for i in range(4)]

            # qc = qT * bcast(c8): c folded into q before the scores; all
            # four head-pairs precomputed so attention never waits on DVE
            qc_t = []
            for hq in range(4):
                cbc_ps = ps.tile([128, S], F32, tag="e2", bufs=2)
                for j in range(2):
                    hp = j * 64
                    h = hq * 2 + j
                    for half in range(2):
                        nc.tensor.matmul(
                            cbc_ps[hp : hp + 64, half * 512 : (half + 1) * 512],
                            lhsT=ones1[0:1, 0:64],
                            rhs=c8row_all[0:1, h * S + half * 512 : h * S + (half + 1) * 512],
                            start=True,
                            stop=True,
                        )
                qc = work.tile([128, S], BF, tag="big4k", bufs=14, name=f"qc{hq}")
                nc.vector.tensor_tensor(
                    out=qc, in0=qT[hq], in1=cbc_ps, op=mybir.AluOpType.mult
                )
                qc_t.append(qc)

            pending = []

            def flush_pending():
                # deferred per-head normalization: by now the reciprocal has
                # long finished, so the rbc matmul never stalls the PE
                while pending:
                    dst_hq, dst_sh, j, avsb, rbf = pending.pop(0)
                    hp = j * 64
                    rbc_ps = ps.tile([128, 512], F32, tag="e", bufs=2)
                    nc.tensor.matmul(rbc_ps, lhsT=ones1, rhs=rbf, start=True, stop=True)
                    rbc_sb = sm.tile([128, 512], BF, tag="rbc", bufs=2)
                    nc.vector.tensor_copy(out=rbc_sb, in_=rbc_ps)
                    nc.vector.tensor_tensor(
                        out=aT[dst_hq][hp : hp + 64, dst_sh * 512 : (dst_sh + 1) * 512],
                        in0=avsb,
                        in1=rbc_sb[0:DHD, :],
                        op=mybir.AluOpType.mult,
                    )

            def do_av(hq, pT2, sh_list=(0, 1)):
                for sh in sh_list:
                    for j in range(2):
                        h = hq * 2 + j
                        av_ps = ps.tile([DHD + 1, 512], F32, tag="pt", bufs=2)
                        for tj in range(8):
                            nc.tensor.matmul(
                                av_ps,
                                lhsT=v_sb[tj][:, h, :],
                                rhs=pT2[j][:, tj, sh * 512 : (sh + 1) * 512],
                                start=(tj == 0),
                                stop=(tj == 7),
                            )
                        # heads to SBUF right away (releases the PSUM bank);
                        # denominator row via fast-approx reciprocal
                        avsb = sm.tile([DHD, 512], BF, tag="avsb", bufs=4)
                        nc.vector.tensor_copy(out=avsb, in_=av_ps[0:DHD, :])
                        rrow_f = sm.tile([1, 512], F32, tag="rrowf", bufs=2)
                        nc.vector.tensor_copy(out=rrow_f, in_=av_ps[DHD : DHD + 1, :])
                        rinv = sm.tile([1, 512], F32, tag="rinv", bufs=2)
                        nc.vector.reciprocal_approx_fast(out=rinv, in_=rrow_f)
                        rbf = sm.tile([1, 512], BF, tag="rbf", bufs=2)
                        nc.vector.tensor_copy(out=rbf, in_=rinv)
                        pending.append((hq, sh, j, avsb, rbf))
                    flush_pending()

            prev = None
            for hq in range(4):
                qc = qc_t[hq]
                pT2 = [
                    work.tile([128, 8, S], BF, tag="pt16k", bufs=4, name=f"pT{j}")
                    for j in range(2)
                ]
                for tt in range(8):
                    for j in range(2):
                        hp = j * 64
                        eT = ps.tile([128, S], F32, tag="e2", bufs=2, name=f"eT{j}")
                        for half in range(2):
                            nc.tensor.matmul(
                                eT[:, half * 512 : (half + 1) * 512],
                                lhsT=kT[hq][hp : hp + 64, tt * 128 : (tt + 1) * 128],
                                rhs=qc[hp : hp + 64, half * 512 : (half + 1) * 512],
                                start=True,
                                stop=True,
                            )
                        nc.scalar.activation(
                            out=pT2[j][:, tt, :], in_=eT, func=AF.Exp,
                            bias=0.0, scale=1.0,
                        )
                if dbg and hq == 0:
                    nc.sync.dma_start(out=dpT, in_=pT2[0])
                if hq == 0:
                    v_proj()
                    if dbg:
                        for i in range(N_ST):
                            nc.sync.dma_start(
                                out=dv[i * 128 : (i + 1) * 128, :, :], in_=v_sb[i]
                            )
                if prev is not None:
                    do_av(*prev)
                prev = (hq, pT2)

            # ---------------- phase 3: WO + residual + LN1 ----------------
            x2f_sb = [work.tile([128, D], F32, tag="v2kf", bufs=8, name=f"x2f_{i}") for i in range(N_ST)]
            x2T = [work.tile([128, S], BF, tag="big4k", bufs=14, name=f"x2T{i}") for i in range(3)]
            nc.vector.memset(x2T[2], 0.0)
            LCORR = float(D) / float(D - 1)

            def layer_norm(dst, src_ps, res_tiles, g_bc, b_bc):
                xr = sm.tile([128, D], F32, tag="xr", bufs=3)
                nc.vector.tensor_add(xr, src_ps, res_tiles[0])
                for rt in res_tiles[1:]:
                    nc.vector.tensor_add(xr, xr, rt)
                stats = sm.tile([128, 6], F32, tag="lstats", bufs=4)
                nc.vector.bn_stats(out=stats, in_=xr)
                mv = sm.tile([128, 2], F32, tag="lmv", bufs=4)
                nc.vector.bn_aggr(out=mv, in_=stats)
                sd = sm.tile([128, 1], F32, tag="lsd", bufs=4)
                nc.scalar.activation(
                    out=sd, in_=mv[:, 1:2], func=AF.Sqrt, bias=eps_l, scale=LCORR
                )
                rstd = sm.tile([128, 1], F32, tag="lrstd", bufs=4)
                nc.vector.reciprocal(out=rstd, in_=sd)
                grstd = sm.tile([128, 1], F32, tag="lgr", bufs=4)
                nc.vector.tensor_mul(grstd, rstd, g_bc)
                nc.vector.tensor_scalar(
                    out=dst,
                    in0=xr,
                    scalar1=mv[:, 0:1],
                    scalar2=grstd,
                    op0=mybir.AluOpType.subtract,
                    op1=mybir.AluOpType.mult,
                )
                nc.vector.tensor_scalar_add(dst, dst, b_bc)

            def wo_ln1(st_list):
                # pass 1: all WO matmuls + LN emissions (PE streams while the
                # DVE LN chains drain); pass 2: the x2 transposes
                for st in st_list:
                    x1_ps = ps.tile([128, D], F32, tag="e", bufs=2)
                    for it in range(4):
                        nc.tensor.matmul(
                            x1_ps,
                            lhsT=aT[it][:, st * 128 : (st + 1) * 128],
                            rhs=wo_sb[it],
                            start=(it == 0),
                            stop=(it == 3),
                        )
                    layer_norm(x2f_sb[st], x1_ps, [x_sb[:, st, :]], g1_bc, b1_bc)
                for st in st_list:
                    xt_ps = ps.tile([128, 4, 128], F32, tag="pt", bufs=2)
                    for jc, (j0, jn) in enumerate([(0, 128), (128, 128), (256, 44)]):
                        nc.tensor.transpose(
                            xt_ps[:jn, jc, :], x2f_sb[st][:, j0 : j0 + jn], identf
                        )
                    for jc, (j0, jn) in enumerate([(0, 128), (128, 128), (256, 44)]):
                        nc.vector.tensor_copy(
                            out=x2T[jc][:jn, st * 128 : (st + 1) * 128],
                            in_=xt_ps[:jn, jc, :],
                        )

            # drain the last head pair one s-half at a time, interleaving the
            # WO/LN1 tiles that become ready
            do_av(prev[0], prev[1], (0,))
            wo_ln1(range(0, 4))
            do_av(prev[0], prev[1], (1,))
            wo_ln1(range(4, 8))

            if dbg:
                for i in range(4):
                    nc.sync.dma_start(out=daT[i * 128 : (i + 1) * 128, :], in_=aT[i])
                for i in range(N_ST):
                    nc.sync.dma_start(out=dx2[i * 128 : (i + 1) * 128, :],
                                      in_=x2f_sb[i])

            # ---------------- phase 4: FFN + LN2 ----------------
            # FFN1(sh) then immediately FFN2+LN2 for that s-half, so the LN2
            # chains overlap the other half's FFN1; relu alternates between
            # ACT and DVE to halve the activation-engine serial time
            h1T = [work.tile([128, S], BF, tag="big4k", bufs=14, name=f"h1T{i}") for i in range(10)]
            def ffn2_ln2(st_list):
                for st in st_list:
                    h2_ps = ps.tile([128, D], F32, tag="e2", bufs=2)
                    for mt, (m0, msz) in enumerate(M_CHUNKS):
                        nc.tensor.matmul(
                            h2_ps,
                            lhsT=h1T[mt][:msz, st * 128 : (st + 1) * 128],
                            rhs=w2_sb[mt][:msz, :],
                            start=(mt == 0),
                            stop=(mt == 9),
                        )
                    o_sb = sm.tile([128, D], F32, tag="o", bufs=2)
                    layer_norm(o_sb, h2_ps, [fb2_bc, x2f_sb[st]], g2_bc, b2_bc)
                    dge[st % 2].dma_start(
                        out=outd[st * 128 : (st + 1) * 128, :], in_=o_sb
                    )

            for sh in range(N_SH):
                for mt, (m0, msz) in enumerate(M_CHUNKS):
                    h1_ps = pstile()
                    for jc, (j0, jn) in enumerate(J_CHUNKS):
                        nc.tensor.matmul(
                            h1_ps[:msz, :],
                            lhsT=w1_sb[jc][:jn, m0 : m0 + msz],
                            rhs=x2T[jc][:jn, sh * 512 : (sh + 1) * 512],
                            start=(jc == 0),
                            stop=(jc == 2),
                        )
                    nc.scalar.activation(
                        out=h1T[mt][:msz, sh * 512 : (sh + 1) * 512],
                        in_=h1_ps[:msz, :],
                        func=AF.Relu,
                        bias=fb1_sb[mt][:msz, :],
                        scale=1.0,
                    )
                if sh == 0:
                    ffn2_ln2(range(0, 4))
            ffn2_ln2(range(4, 8))
            if dbg:
                for mt, (m0, msz) in enumerate(M_CHUNKS):
                    nc.sync.dma_start(out=dh1[m0 : m0 + msz, :], in_=h1T[mt][:msz, :])

    nc.compile()
    return nc


def _get_nc():
    if "nc" not in _cache:
        _cache["nc"] = _build_nc()
    return _cache["nc"]


def prep_in_maps(x, WQ, WK, WV, WO, W1, b1, W2, b2, gamma_a, beta_a,
                 gamma1, beta1, gamma2, beta2):
    f = np.float32
    x = np.asarray(x, f)

    def perm(W):
        # head h -> contiguous rows [h*64, (h+1)*64)
        return np.asarray(W, f).reshape(DHD, H, D).transpose(1, 0, 2).reshape(DH, D)

    def padr(a, rows, cols=None):
        out = np.zeros((rows, cols or a.shape[1]), f)
        out[: a.shape[0], : a.shape[1]] = a
        return out

    def tobf(a):
        return np.ascontiguousarray(a.astype(NPBF))

    wq_t = padr(perm(WQ).T, DP)
    wk_t = padr(perm(WK).T, DP)
    wv_t = padr(perm(WV).T, DP)
    w1 = padr(np.asarray(W1, f), DP, DFP)
    # bigm: [wo rows as 4 col-blocks | w2 rows as 10 col-blocks] on 128 rows
    wo4 = np.asarray(WO, f).reshape(4, 128, D).transpose(1, 0, 2).reshape(128, 4 * D)
    w2p = padr(np.asarray(W2, f), DFP)
    w210 = w2p.reshape(10, 128, D).transpose(1, 0, 2).reshape(128, 10 * D)
    bigm = tobf(np.concatenate([wo4, w210], axis=1))
    b1p = np.zeros(1280, f)
    b1p[:DF] = np.asarray(b1, f)
    fb1 = np.ascontiguousarray(b1p.reshape(10, 128).T)
    fb2 = np.ascontiguousarray(np.asarray(b2, f))
    # beta_a drops out of softmax (per-row constant shift); the 1/sqrt(D)
    # score scale cancels inside the score LayerNorm: softmax(g*LN(e/sqrt(D)))
    # == softmax(g/sqrt(var(e) + D*eps) * e), so gamma is used unscaled and
    # D*eps replaces eps on-device.
    gal = np.concatenate([
        np.asarray(gamma_a, f).reshape(H),
        np.asarray([gamma1, beta1, gamma2, beta2], f),
    ]).astype(f)

    bigw1 = tobf(np.concatenate([wk_t, wv_t, w1], axis=1))
    shared = {"bigw1": bigw1, "bigm": bigm, "fb1": fb1, "fb2": fb2, "gal": gal}
    in_maps = []
    for b in range(B):
        xb = np.ascontiguousarray(x[b])
        xt = padr(np.ascontiguousarray(xb.T), DP)
        bigw0 = tobf(np.concatenate([xt, wq_t], axis=1))
        in_maps.append({"x": xb, "bigw0": bigw0, **shared})
    return in_maps


def kernel(**inputs):
    global _last_results
    in_maps = prep_in_maps(**inputs)
    nc = _get_nc()
    res = run_bass_kernel_spmd(nc, in_maps, core_ids=list(range(NCORES)), trace=TRACE)
    _last_results = res
    return np.stack([res.results[b]["out"] for b in range(B)], axis=0)
